# revision 32
# baseline (speedup 1.0000x reference)
"""Trainium2 Bass kernel for nn_Encoder_51582557225692 (sparse_attention).

Key facts exploited:
  * The reference appends beacon states BEFORE each of the L=2 layers and
    returns only those, so the output is [beacon_input, beacon_after_layer0].
    Layer 1 never affects the output; only the beacon half of layer 0's
    output is needed.
  * Sharding: tensor-parallel over the 32 heads (4 per core) for attention
    and over the MLP intermediate dim (1376 per core, padded to 1408) across
    the 8 NeuronCores. One on-device fp32 AllReduce produces the
    post-attention residual x_b (needed in full on every core for rms2);
    the row-sharded MLP down-projection partials are summed on the host
    during unsharding.
  * Activations are kept feature-major ([feature, token]) on chip so every
    projection consumes weights in their natural [in, out] layout as the
    stationary matmul operand; V is produced token-major directly by
    swapping the matmul operand roles. Matmul inputs are bf16 (fp32
    accumulation in PSUM); softmax/rope/residual math is fp32.

Self-contained: hardcodes shapes; host-side prep shards/casts/tiles inputs.
"""
import numpy as np
import ml_dtypes

import concourse.bass as bass
import concourse.bacc as bacc
import concourse.mybir as mybir
import concourse.tile as tile
from concourse.bass_utils import run_bass_kernel_spmd

# problem dims
B, S, H, NH, HD = 4, 64, 4096, 32, 128
I = 11008
NCORES = 8
HPC = H // NCORES            # 512 per-core attention feature slice (4 heads)
MT = HPC // 128              # 4 head tiles per core
KT = H // 128                # 32 contraction tiles over H
IPC = I // NCORES            # 1376
GT = 11                      # padded I tiles per core (1408 = 11*128)
IPC_PAD = GT * 128
T2 = 2 * B * S               # 512 tokens (ord + bea), b-major within halves
TB = B * S                   # 256 beacon (or ordinal, or memory) tokens
EPS = 1e-5
SCALE = float(1.0 / np.sqrt(HD))

F32 = mybir.dt.float32
BF16 = mybir.dt.bfloat16
AX = mybir.AxisListType
ALU = mybir.AluOpType
ACTF = mybir.ActivationFunctionType

_CACHE = {}


def _build_nc():
    nc = bacc.Bacc("TRN2", target_bir_lowering=False, debug=False,
                   num_devices=NCORES)
    d = {}
    def din(name, shape, dt=BF16):
        d[name] = nc.dram_tensor(name, list(shape), dt, kind="ExternalInput").ap()
    def dout(name, shape, dt=F32):
        d[name] = nc.dram_tensor(name, list(shape), dt, kind="ExternalOutput").ap()

    din('x_d', (128, KT, T2))            # bf16 feature-major [p, k, tok]
    din('mem_d', (128, KT, TB))
    din('b8_d', (128, KT, TB), F32)      # beacon/8 feature-major
    din('cosq_d', (128, TB), F32)
    din('sinq_d', (128, TB), F32)
    din('cosk_d', (128, T2), F32)
    din('sink_d', (128, T2), F32)
    din('cosm_d', (128, TB), F32)
    din('sinm_d', (128, TB), F32)
    din('mask_d', (64, 192), F32)
    din('ones_d', (128, 1), F32)
    din('onesr_d', (1, 128), F32)
    din('ident_d', (64, 64))
    din('wq_d', (128, MT, KT, 128))
    din('wk_d', (128, MT, KT, 128))
    din('wkm_d', (128, MT, KT, 128))
    din('wv_d', (128, 8, 4, HPC))
    din('wvm_d', (128, 8, 4, HPC))
    din('wo_d', (128, KT, MT, 128))
    din('wg_d', (128, GT, KT, 128))
    din('wu_d', (128, GT, KT, 128))
    din('wd_d', (128, KT, GT, 128))
    dout('oxb_d', (128, KT, TB))
    dout('od_d', (128, KT, TB))

    with tile.TileContext(nc) as tc:
        _emit(nc, tc, d)
    nc.compile()
    return nc


def _emit(nc, tc, d):
    sync, vec, sca, gps, ten = nc.sync, nc.vector, nc.scalar, nc.gpsimd, nc.tensor

    with tc.tile_pool(name="const", bufs=1) as pconst, \
         tc.tile_pool(name="acts", bufs=1) as pacts, \
         tc.tile_pool(name="dram", bufs=1, space="DRAM") as pdram:

        # ---- constants / inputs resident in SBUF
        cosq = pconst.tile([128, TB], F32, name="cosq"); sync.dma_start(cosq[:], d['cosq_d'])
        sinq = pconst.tile([128, TB], F32, name="sinq"); sync.dma_start(sinq[:], d['sinq_d'])
        cosk = pconst.tile([128, T2], F32, name="cosk"); sync.dma_start(cosk[:], d['cosk_d'])
        sink = pconst.tile([128, T2], F32, name="sink"); sync.dma_start(sink[:], d['sink_d'])
        cosm = pconst.tile([128, TB], F32, name="cosm"); sync.dma_start(cosm[:], d['cosm_d'])
        sinm = pconst.tile([128, TB], F32, name="sinm"); sync.dma_start(sinm[:], d['sinm_d'])
        maskt = pconst.tile([64, 192], F32, name="maskt"); sync.dma_start(maskt[:], d['mask_d'])
        ones = pconst.tile([128, 1], F32, name="ones"); sync.dma_start(ones[:], d['ones_d'])
        onesr = pconst.tile([1, 128], F32, name="onesr"); sync.dma_start(onesr[:], d['onesr_d'])
        ident = pconst.tile([64, 64], BF16, name="ident"); sync.dma_start(ident[:], d['ident_d'])
        rv1 = pconst.tile([1, T2], F32)
        rv1b = pconst.tile([128, T2], F32)
        rv1t = pconst.tile([128, 4], F32)
        rv2b = pconst.tile([128, TB], F32)
        eps1 = pconst.tile([1, 1], F32)
        vec.memset(eps1[:], EPS)

        # ---- persistent activations
        kTs = pacts.tile([128, MT, T2], BF16)     # roped+normed K (feature-major)
        qTs = pacts.tile([128, MT, TB], BF16)
        mkTs = pacts.tile([128, MT, TB], BF16)
        # token-major V pieces, base-0 per-batch layout [tok(64), b, feat]
        ordv2 = pacts.tile([64, B, HPC], BF16)
        beav2 = pacts.tile([64, B, HPC], BF16)
        memv2 = pacts.tile([64, B, HPC], BF16)
        oTs = pacts.tile([128, MT, TB], BF16)     # attention out (feature-major)

        # chunk-contiguous layout [k, p, t] so each AllReduce chunk is contiguous
        NCH = 4                      # AllReduce chunks
        KCH = KT // NCH
        cc_in = pdram.tile([KT, 128, TB], F32)
        cc_outs = [pdram.tile([KCH, 128, TB], F32, addr_space="Shared",
                              name=f"cc_out{c}") for c in range(NCH)]

        # ================= phase 0: load x/mem, rms1 stats =================
        with tc.tile_pool(name="px", bufs=1) as px, \
             tc.tile_pool(name="pp_ss", bufs=1, space="PSUM") as pp_ss, \
             tc.tile_pool(name="psq", bufs=3) as psq:
            xt = px.tile([128, KT, T2], BF16)
            memt = px.tile([128, KT, TB], BF16)
            for c in range(4):
                kc = slice(c * 8, (c + 1) * 8)
                sync.dma_start(xt[:, kc], d['x_d'][:, kc])
                sync.dma_start(memt[:, kc], d['mem_d'][:, kc])

            ss1 = pp_ss.tile([1, T2], F32)
            for k in range(KT):
                sq = psq.tile([128, T2], F32, name="sq", tag="sq")
                vec.tensor_tensor(sq[:], xt[:, k], xt[:, k], op=ALU.mult)
                ten.matmul(ss1[:], ones[:], sq[:], start=(k == 0), stop=(k == KT - 1))
            sd1 = pconst.tile([1, T2], F32)
            sca.activation(sd1[:], ss1[:], ACTF.Sqrt, bias=eps1[0:1, 0:1], scale=1.0 / H)
            vec.reciprocal(rv1[:], sd1[:])
            bc1 = pp_ss.tile([128, T2], F32, name="bc1", tag="bc1")
            ten.matmul(bc1[:], onesr[:], rv1[:])
            vec.tensor_copy(rv1b[:], bc1[:])
            # token-major rv1 for scaling V: rv1t[p, m] = rv1[0, m*128+p]
            for m in range(4):
                sync.dma_start(rv1t[:, m:m + 1], rv1[0:1, m * 128:(m + 1) * 128])

            # ================= phase 1: V projections (token-major) ========
            with tc.tile_pool(name="pvst", bufs=1) as pvst, \
                 tc.tile_pool(name="pwv", bufs=3) as pwv, \
                 tc.tile_pool(name="pp_v", bufs=1, space="PSUM") as pp_v:
                ordv = pvst.tile([128, 2, HPC], BF16)
                beav = pvst.tile([128, 2, HPC], BF16)
                memv = pvst.tile([128, 2, HPC], BF16)
                vps = [pp_v.tile([128, HPC], F32, name=f"vps{j}", tag=f"vps{j}")
                       for j in range(4)]
                for kg in range(8):
                    wvt = pwv.tile([128, 4, HPC], BF16, name="wvt", tag="wv")
                    sync.dma_start(wvt[:], d['wv_d'][:, kg])
                    for kk in range(4):
                        k = kg * 4 + kk
                        st, sp = (k == 0), (k == KT - 1)
                        ten.matmul(vps[0][:], xt[:, k, 0:128], wvt[:, kk], start=st, stop=sp)
                        ten.matmul(vps[1][:], xt[:, k, 128:256], wvt[:, kk], start=st, stop=sp)
                        ten.matmul(vps[2][:], xt[:, k, 256:384], wvt[:, kk], start=st, stop=sp)
                        ten.matmul(vps[3][:], xt[:, k, 384:512], wvt[:, kk], start=st, stop=sp)
                for j in range(2):
                    vec.tensor_scalar_mul(ordv[:, j], vps[j][:], rv1t[:, j:j + 1])
                    vec.tensor_scalar_mul(beav[:, j], vps[2 + j][:], rv1t[:, 2 + j:3 + j])
                with tc.tile_pool(name="pwvm", bufs=3) as pwvm, \
                     tc.tile_pool(name="pp_vm", bufs=1, space="PSUM") as pp_vm:
                    mps = [pp_vm.tile([128, HPC], F32, name=f"mps{j}", tag=f"mps{j}")
                           for j in range(2)]
                    for kg in range(8):
                        wvmt = pwvm.tile([128, 4, HPC], BF16, name="wvmt", tag="wvm")
                        sync.dma_start(wvmt[:], d['wvm_d'][:, kg])
                        for kk in range(4):
                            k = kg * 4 + kk
                            st, sp = (k == 0), (k == KT - 1)
                            ten.matmul(mps[0][:], memt[:, k, 0:128], wvmt[:, kk], start=st, stop=sp)
                            ten.matmul(mps[1][:], memt[:, k, 128:256], wvmt[:, kk], start=st, stop=sp)
                    for j in range(2):
                        vec.tensor_copy(memv[:, j], mps[j][:])
                # rearrange to base-0 per-batch layout: v2[s, b, f] =
                # v[64*(b%2)+s, b//2, f]  (one SBUF->SBUF DMA each)
                for src_t, dst_t in ((ordv, ordv2), (beav, beav2), (memv, memv2)):
                    for hh in range(2):
                        sync.dma_start(
                            dst_t[:].rearrange("s (j h) f -> s j h f", h=2)[:, :, hh],
                            src_t[64 * hh:64 * (hh + 1)])

            # ========== phase 2: K/Q/MK projections + rope + normalize =====
            with tc.tile_pool(name="pwl", bufs=3) as pwl, \
                 tc.tile_pool(name="prope", bufs=2) as pr, \
                 tc.tile_pool(name="pp_k", bufs=2, space="PSUM") as pp_k:
                for m in range(MT):
                    for src, wdram, cos_t, sin_t, ncols, coloff, dst, norm in (
                            ('x', 'wk_d', cosk, sink, T2, 0, kTs, True),
                            ('x', 'wq_d', cosq, sinq, TB, TB, qTs, True),
                            ('m', 'wkm_d', cosm, sinm, TB, 0, mkTs, False)):
                        wt = pwl.tile([128, KT, 128], BF16, name="wt", tag="wl")
                        sync.dma_start(wt[:], d[wdram][:, m])
                        ps = pp_k.tile([128, ncols], F32, name="kps", tag="kps")
                        for k in range(KT):
                            rhs = xt[:, k, coloff:coloff + ncols] if src == 'x' \
                                else memt[:, k]
                            ten.matmul(ps[:], wt[:, k], rhs, start=(k == 0),
                                       stop=(k == KT - 1))
                        raw = pr.tile([128, ncols], F32, name="raw", tag="raw")
                        vec.tensor_copy(raw[:], ps[:])
                        rot = pr.tile([128, ncols], F32, name="rot", tag="rot")
                        sync.dma_start(rot[0:64, :], raw[64:128, :])
                        sync.dma_start(rot[64:128, :], raw[0:64, :])
                        t1 = pr.tile([128, ncols], F32, name="t1", tag="t1")
                        vec.tensor_tensor(t1[:], rot[:], sin_t[:], op=ALU.mult)
                        t2 = pr.tile([128, ncols], F32, name="t2", tag="t2")
                        vec.tensor_tensor(t2[:], raw[:], cos_t[:], op=ALU.mult)
                        if norm:
                            t3 = pr.tile([128, ncols], F32, name="t3", tag="t3")
                            vec.tensor_tensor(t3[:], t1[:], t2[:], op=ALU.add)
                            rvsl = rv1b[:, coloff:coloff + ncols] if ncols != T2 \
                                else rv1b[:]
                            vec.tensor_tensor(dst[:, m], t3[:], rvsl, op=ALU.mult)
                        else:
                            vec.tensor_tensor(dst[:, m], t1[:], t2[:], op=ALU.add)

        # ====== phases 3+4: attention (head-outer), per-head Wo accumulate,
        # ====== chunked AllReduce overlapping the attention tail
        with tc.tile_pool(name="acts2", bufs=1) as pacts2:
            xacc = pacts2.tile([128, KT, TB], F32)    # Wo partial + beacon/8
            xbs = pacts2.tile([128, KT, TB], F32)     # post-AR residual
            h2s = pacts2.tile([128, KT, TB], BF16)
            h3s = pacts2.tile([128, GT, TB], BF16)
            sync.dma_start(xacc[:], d['b8_d'])        # init with beacon/8
            att_block = [
                tc.tile_pool(name="patt", bufs=3),
                tc.tile_pool(name="pwo", bufs=1),
                tc.tile_pool(name="pp_sc", bufs=2, space="PSUM"),
                tc.tile_pool(name="pp_tr", bufs=1, space="PSUM"),
                tc.tile_pool(name="pp_ot", bufs=1, space="PSUM"),
                tc.tile_pool(name="pp_wo", bufs=2, space="PSUM"),
            ]
            pa, pwo, pp_sc, pp_tr, pp_ot, pp_wo = [p.__enter__() for p in att_block]
            wo_all = pwo.tile([128, KT, MT, 128], BF16)
            sync.dma_start(wo_all[:], d['wo_d'])
            for h in range(MT):
                hc = 128 * h
                for b in range(B):
                    bo = 64 * b
                    sc_ps = pp_sc.tile([64, 192], F32, name="sc_ps", tag="sc")
                    q_ap = qTs[:, h, bo:bo + 64]
                    ten.matmul(sc_ps[:, 0:64], q_ap, mkTs[:, h, bo:bo + 64])
                    ten.matmul(sc_ps[:, 64:128], q_ap, kTs[:, h, bo:bo + 64])
                    ten.matmul(sc_ps[:, 128:192], q_ap, kTs[:, h, TB + bo:TB + bo + 64])
                    sc = pa.tile([64, 192], F32, name="sc", tag="scs")
                    vec.scalar_tensor_tensor(sc[:], sc_ps[:], SCALE, maskt[:],
                                             op0=ALU.mult, op1=ALU.add)
                    mx = pa.tile([64, 1], F32, name="mx", tag="mx")
                    vec.reduce_max(mx[:], sc[:], axis=AX.X)
                    mxn = pa.tile([64, 1], F32, name="mxn", tag="mxn")
                    vec.tensor_scalar_mul(mxn[:], mx[:], -1.0)
                    pex = pa.tile([64, 192], F32, name="pex", tag="pex")
                    rs = pa.tile([64, 1], F32, name="rs", tag="rs")
                    sca.activation(pex[:], sc[:], ACTF.Exp, bias=mxn[:, 0:1],
                                   accum_out=rs[:, 0:1])
                    rr = pa.tile([64, 1], F32, name="rr", tag="rr")
                    vec.reciprocal(rr[:], rs[:])
                    pbf = pa.tile([64, 192], BF16, name="pbf", tag="pbf")
                    vec.tensor_scalar_mul(pbf[:], pex[:], rr[:, 0:1])
                    trm = pp_tr.tile([64, 64], BF16, name="trm", tag="trm")
                    ten.transpose(trm[:], pbf[:, 0:64], ident[:])
                    tro = pp_tr.tile([64, 64], BF16, name="tro", tag="tro")
                    ten.transpose(tro[:], pbf[:, 64:128], ident[:])
                    trb = pp_tr.tile([64, 64], BF16, name="trb", tag="trb")
                    ten.transpose(trb[:], pbf[:, 128:192], ident[:])
                    ptm = pa.tile([64, 64], BF16, name="ptm", tag="ptm")
                    vec.tensor_copy(ptm[:], trm[:])
                    pto = pa.tile([64, 64], BF16, name="pto", tag="pto")
                    vec.tensor_copy(pto[:], tro[:])
                    ptb = pa.tile([64, 64], BF16, name="ptb", tag="ptb")
                    vec.tensor_copy(ptb[:], trb[:])
                    o_ps = pp_ot.tile([128, 64], F32, name="o_ps", tag="ops")
                    ten.matmul(o_ps[:], memv2[:, b, hc:hc + 128],
                               ptm[:], start=True, stop=False)
                    ten.matmul(o_ps[:], ordv2[:, b, hc:hc + 128],
                               pto[:], start=False, stop=False)
                    ten.matmul(o_ps[:], beav2[:, b, hc:hc + 128],
                               ptb[:], start=False, stop=True)
                    vec.tensor_copy(oTs[:, h, bo:bo + 64], o_ps[:])
                # this head's Wo contribution into xacc (all 32 m-tiles)
                for m in range(KT):
                    wps = pp_wo.tile([128, TB], F32, name="wps", tag="wps")
                    ten.matmul(wps[:], wo_all[:, m, h, :], oTs[:, h, :])
                    vec.tensor_tensor(xacc[:, m], wps[:], xacc[:, m], op=ALU.add)
                    if h == MT - 1:
                        sync.dma_start(cc_in[m], xacc[:, m])
                        if (m + 1) % KCH == 0:
                            c = m // KCH
                            cs = slice(c * KCH, (c + 1) * KCH)
                            gps.collective_compute(
                                "AllReduce", ALU.add,
                                replica_groups=[list(range(NCORES))],
                                ins=[cc_in[cs].opt()], outs=[cc_outs[c].opt()])
                            sync.dma_start(
                                xbs[:, cs],
                                cc_outs[c][:].rearrange("k p t -> p k t"))
                            sync.dma_start(d['oxb_d'][:, cs], xbs[:, cs])
            for p in reversed(att_block):
                p.__exit__(None, None, None)

            # =============== phase 5: rms2 + h2 (chunk-pipelined) ==========
            with tc.tile_pool(name="psq2", bufs=3) as psq2, \
                 tc.tile_pool(name="pp_s2", bufs=1, space="PSUM") as pp_s2:
                ss2 = pp_s2.tile([1, TB], F32)
                for k in range(KT):
                    sq2 = psq2.tile([128, TB], F32, name="sq2", tag="sq2")
                    vec.tensor_tensor(sq2[:], xbs[:, k], xbs[:, k], op=ALU.mult)
                    ten.matmul(ss2[:], ones[:], sq2[:], start=(k == 0),
                               stop=(k == KT - 1))
                sd2 = pconst.tile([1, TB], F32)
                sca.activation(sd2[:], ss2[:], ACTF.Sqrt, bias=eps1[0:1, 0:1],
                               scale=1.0 / H)
                rv2 = pconst.tile([1, TB], F32)
                vec.reciprocal(rv2[:], sd2[:])
                bc2 = pp_s2.tile([128, TB], F32, name="bc2", tag="bc2")
                ten.matmul(bc2[:], onesr[:], rv2[:])
                vec.tensor_copy(rv2b[:], bc2[:])
                for k in range(KT):
                    vec.tensor_tensor(h2s[:, k], xbs[:, k], rv2b[:], op=ALU.mult)

            # =============== phase 6: MLP gate/up ============================
            with tc.tile_pool(name="pwg", bufs=4) as pwg, \
                 tc.tile_pool(name="pgu", bufs=2) as pgu, \
                 tc.tile_pool(name="pp_gu", bufs=2, space="PSUM") as pp_gu:
                for m in range(GT):
                    wgt = pwg.tile([128, KT, 128], BF16, name="wgt", tag="wgu")
                    sync.dma_start(wgt[:], d['wg_d'][:, m])
                    wut = pwg.tile([128, KT, 128], BF16, name="wut", tag="wgu")
                    sync.dma_start(wut[:], d['wu_d'][:, m])
                    gps_t = pp_gu.tile([128, TB], F32, name="gps_t", tag="gps")
                    ups_t = pp_gu.tile([128, TB], F32, name="ups_t", tag="ups")
                    for k in range(KT):
                        st, sp = (k == 0), (k == KT - 1)
                        ten.matmul(gps_t[:], wgt[:, k], h2s[:, k], start=st, stop=sp)
                        ten.matmul(ups_t[:], wut[:, k], h2s[:, k], start=st, stop=sp)
                    # silu(g) * u = g * sigmoid(g) * u  (sim lacks Silu)
                    gsig = pgu.tile([128, TB], F32, name="gsig", tag="gsig")
                    sca.activation(gsig[:], gps_t[:], ACTF.Sigmoid)
                    gact = pgu.tile([128, TB], BF16, name="gact", tag="gact")
                    vec.tensor_tensor(gact[:], gps_t[:], gsig[:], op=ALU.mult)
                    ubf = pgu.tile([128, TB], BF16, name="ubf", tag="ubf")
                    vec.tensor_copy(ubf[:], ups_t[:])
                    vec.tensor_tensor(h3s[:, m], gact[:], ubf[:], op=ALU.mult)

            # =============== phase 7: MLP down (partial) =====================
            with tc.tile_pool(name="pwd", bufs=3) as pwd, \
                 tc.tile_pool(name="pds", bufs=3) as pds, \
                 tc.tile_pool(name="pp_d", bufs=2, space="PSUM") as pp_d:
                for m in range(KT):
                    wdt = pwd.tile([128, GT, 128], BF16, name="wdt", tag="wd")
                    sync.dma_start(wdt[:], d['wd_d'][:, m])
                    dps = pp_d.tile([128, TB], F32, name="dps", tag="dps")
                    for k in range(GT):
                        ten.matmul(dps[:], wdt[:, k], h3s[:, k], start=(k == 0),
                                   stop=(k == GT - 1))
                    dst = pds.tile([128, TB], F32, name="dst", tag="dst")
                    vec.tensor_copy(dst[:], dps[:])
                    sync.dma_start(d['od_d'][:, m], dst[:])


# ======================= host-side prep / unshard ==========================

def _bf16(x):
    return np.ascontiguousarray(np.asarray(x, np.float32).astype(ml_dtypes.bfloat16))


def _fm_tiles(fm):
    """[H, T] feature-major -> [128, KT, T] partition-major tiles."""
    T = fm.shape[1]
    return np.ascontiguousarray(fm.reshape(KT, 128, T).transpose(1, 0, 2))


def _lhsT_tiles(w, mt):
    """[H_in, M] weight -> [128, mt, H_in//128, 128] lhsT tiles."""
    hi, mo = w.shape
    kt = hi // 128
    return np.ascontiguousarray(
        w.reshape(kt, 128, mt, 128).transpose(1, 2, 0, 3))


def _host_prep(inputs):
    ids = np.asarray(inputs['input_ids'])
    embed = np.asarray(inputs['embed'], np.float32)
    beacon = np.asarray(inputs['beacon'], np.float32)
    memory = np.asarray(inputs['memory'], np.float32)
    hs = embed[ids]                                     # [B,S,H]
    x_fm = np.concatenate([hs.reshape(TB, H).T, beacon.reshape(TB, H).T], axis=1)
    mem_fm = memory[0:B].reshape(TB, H).T
    b8_fm = (beacon.reshape(TB, H).T / 8.0).astype(np.float32)

    inv = 1.0 / (10000.0 ** (np.arange(0, HD, 2, dtype=np.float32) / HD))
    f = np.outer(np.arange(3 * S, dtype=np.float32), inv)
    emb = np.concatenate([f, f], axis=-1)               # [3S, HD]
    cos = np.cos(emb).astype(np.float32)
    sin = np.sin(emb).astype(np.float32)

    def fm_tab(p0):
        c = cos[p0:p0 + S].T                            # [HD, S]
        s = sin[p0:p0 + S].T
        s = np.concatenate([-s[:64], s[64:]], axis=0)   # sign-baked for rotate64
        return np.tile(c, (1, B)), np.tile(s, (1, B))
    cosm_t, sinm_t = fm_tab(0)
    coso_t, sino_t = fm_tab(S)
    cosb_t, sinb_t = fm_tab(2 * S)
    cosk_t = np.concatenate([coso_t, cosb_t], axis=1)
    sink_t = np.concatenate([sino_t, sinb_t], axis=1)

    # mask tile columns are [mem(64) | ord(64) | bea(64)] = 192, additive
    maskadd = np.zeros((S, 3 * S), np.float32)
    ii = np.arange(S)[:, None]
    jj = np.arange(S)[None, :]
    bea_block = np.where(jj > ii, np.float32(-1e30), np.float32(0.0))
    maskadd[:, 2 * S:] = bea_block

    ln1 = np.asarray(inputs['ln1'], np.float32)[0]
    ln2 = np.asarray(inputs['ln2'], np.float32)[0]
    Wq = np.asarray(inputs['Wq'], np.float32)[0] * ln1[:, None]
    Wk = np.asarray(inputs['Wk'], np.float32)[0] * ln1[:, None]
    Wv = np.asarray(inputs['Wv'], np.float32)[0] * ln1[:, None]
    Wkm = np.asarray(inputs['Wkm'], np.float32)[0]
    Wvm = np.asarray(inputs['Wvm'], np.float32)[0]
    Wo = np.asarray(inputs['Wo'], np.float32)[0]
    Wg = np.asarray(inputs['Wg'], np.float32)[0] * ln2[:, None]
    Wu = np.asarray(inputs['Wu'], np.float32)[0] * ln2[:, None]
    Wd = np.asarray(inputs['Wd'], np.float32)[0]

    shared = {
        'x_d': _fm_tiles(_bf16(x_fm)),
        'mem_d': _fm_tiles(_bf16(mem_fm)),
        'b8_d': np.ascontiguousarray(_fm_tiles(b8_fm)),
        'cosq_d': np.ascontiguousarray(cosb_t), 'sinq_d': np.ascontiguousarray(sinb_t),
        'cosk_d': np.ascontiguousarray(cosk_t), 'sink_d': np.ascontiguousarray(sink_t),
        'cosm_d': np.ascontiguousarray(cosm_t), 'sinm_d': np.ascontiguousarray(sinm_t),
        'mask_d': maskadd,
        'ones_d': np.ones((128, 1), np.float32),
        'onesr_d': np.ones((1, 128), np.float32),
        'ident_d': _bf16(np.eye(64, dtype=np.float32)),
    }

    in_maps = []
    for c in range(NCORES):
        sl = slice(c * HPC, (c + 1) * HPC)
        isl = slice(c * IPC, (c + 1) * IPC)
        wg = np.zeros((H, IPC_PAD), np.float32); wg[:, :IPC] = Wg[:, isl]
        wu = np.zeros((H, IPC_PAD), np.float32); wu[:, :IPC] = Wu[:, isl]
        wd = np.zeros((IPC_PAD, H), np.float32); wd[:IPC] = Wd[isl]
        wv_c = _bf16(Wv[:, sl])
        wvm_c = _bf16(Wvm[:, sl])
        m = dict(shared)
        m['wq_d'] = _lhsT_tiles(_bf16(Wq[:, sl]), MT)
        m['wk_d'] = _lhsT_tiles(_bf16(Wk[:, sl]), MT)
        m['wkm_d'] = _lhsT_tiles(_bf16(Wkm[:, sl]), MT)
        m['wv_d'] = np.ascontiguousarray(
            wv_c.reshape(8, 4, 128, HPC).transpose(2, 0, 1, 3))
        m['wvm_d'] = np.ascontiguousarray(
            wvm_c.reshape(8, 4, 128, HPC).transpose(2, 0, 1, 3))
        m['wo_d'] = np.ascontiguousarray(
            _bf16(Wo[sl, :]).reshape(MT, 128, KT, 128).transpose(1, 2, 0, 3))
        m['wg_d'] = _lhsT_tiles(_bf16(wg), GT)
        m['wu_d'] = _lhsT_tiles(_bf16(wu), GT)
        m['wd_d'] = np.ascontiguousarray(
            _bf16(wd).reshape(GT, 128, KT, 128).transpose(1, 2, 0, 3))
        in_maps.append(m)
    return in_maps, beacon


def kernel(**inputs):
    in_maps, beacon = _host_prep(inputs)
    if 'nc' not in _CACHE:
        _CACHE['nc'] = _build_nc()
    nc = _CACHE['nc']
    res = run_bass_kernel_spmd(nc, in_maps, core_ids=list(range(NCORES)))
    _CACHE['last_res'] = res
    results = res.results
    xb = results[0]['oxb_d'].astype(np.float32)          # [128, KT, TB]
    dsum = np.sum([r['od_d'] for r in results], axis=0, dtype=np.float32)
    bs_fm = (xb + dsum).transpose(1, 0, 2).reshape(H, TB)  # [H, 256]
    out = np.empty((2 * B, S, H), np.float32)
    out[:B] = np.asarray(inputs['beacon'], np.float32)
    out[B:] = bs_fm.T.reshape(B, S, H)
    return out


# revision 33
# speedup vs baseline: 1.0904x; 1.0904x over previous
"""Trainium2 Bass kernel for nn_Encoder_51582557225692 (sparse_attention).

Key facts exploited:
  * The reference appends beacon states BEFORE each of the L=2 layers and
    returns only those, so the output is [beacon_input, beacon_after_layer0].
    Layer 1 never affects the output; only the beacon half of layer 0's
    output is needed.
  * Sharding: tensor-parallel over the 32 heads (4 per core) for attention
    and over the MLP intermediate dim (1376 per core, padded to 1408) across
    the 8 NeuronCores. One on-device fp32 AllReduce produces the
    post-attention residual x_b (needed in full on every core for rms2);
    the row-sharded MLP down-projection partials are summed on the host
    during unsharding.
  * Activations are kept feature-major ([feature, token]) on chip so every
    projection consumes weights in their natural [in, out] layout as the
    stationary matmul operand; V is produced token-major directly by
    swapping the matmul operand roles. Matmul inputs are bf16 (fp32
    accumulation in PSUM); softmax/rope/residual math is fp32.

Self-contained: hardcodes shapes; host-side prep shards/casts/tiles inputs.
"""
import numpy as np
import ml_dtypes

import concourse.bass as bass
import concourse.bacc as bacc
import concourse.mybir as mybir
import concourse.tile as tile
from concourse.bass_utils import run_bass_kernel_spmd

# problem dims
B, S, H, NH, HD = 4, 64, 4096, 32, 128
I = 11008
NCORES = 8
HPC = H // NCORES            # 512 per-core attention feature slice (4 heads)
MT = HPC // 128              # 4 head tiles per core
KT = H // 128                # 32 contraction tiles over H
IPC = I // NCORES            # 1376
GT = 11                      # padded I tiles per core (1408 = 11*128)
IPC_PAD = GT * 128
T2 = 2 * B * S               # 512 tokens (ord + bea), b-major within halves
TB = B * S                   # 256 beacon (or ordinal, or memory) tokens
EPS = 1e-5
SCALE = float(1.0 / np.sqrt(HD))

F32 = mybir.dt.float32
BF16 = mybir.dt.bfloat16
AX = mybir.AxisListType
ALU = mybir.AluOpType
ACTF = mybir.ActivationFunctionType

_CACHE = {}


def _build_nc():
    nc = bacc.Bacc("TRN2", target_bir_lowering=False, debug=False,
                   num_devices=NCORES)
    d = {}
    def din(name, shape, dt=BF16):
        d[name] = nc.dram_tensor(name, list(shape), dt, kind="ExternalInput").ap()
    def dout(name, shape, dt=F32):
        d[name] = nc.dram_tensor(name, list(shape), dt, kind="ExternalOutput").ap()

    din('x_d', (128, KT, T2))            # bf16 feature-major [p, k, tok]
    din('mem_d', (128, KT, TB))
    din('b8_d', (128, KT, TB), F32)      # beacon/8 feature-major
    din('cosq_d', (128, TB), F32)
    din('sinq_d', (128, TB), F32)
    din('cosk_d', (128, T2), F32)
    din('sink_d', (128, T2), F32)
    din('cosm_d', (128, TB), F32)
    din('sinm_d', (128, TB), F32)
    din('mask_d', (64, 192), F32)
    din('ones_d', (128, 1), F32)
    din('onesr_d', (1, 128), F32)
    din('ident_d', (64, 64))
    din('wq_d', (128, MT, KT, 128))
    din('wk_d', (128, MT, KT, 128))
    din('wkm_d', (128, MT, KT, 128))
    din('wv_d', (128, 8, 4, HPC))
    din('wvm_d', (128, 8, 4, HPC))
    din('wo_d', (128, KT, MT, 128))
    din('wg_d', (128, GT, KT, 128))
    din('wu_d', (128, GT, KT, 128))
    din('wd_d', (128, KT, GT, 128))
    dout('oxb_d', (128, KT, TB), BF16)
    dout('od_d', (128, KT, TB))

    with tile.TileContext(nc) as tc:
        _emit(nc, tc, d)
    nc.compile()
    return nc


def _emit(nc, tc, d):
    sync, vec, sca, gps, ten = nc.sync, nc.vector, nc.scalar, nc.gpsimd, nc.tensor

    with tc.tile_pool(name="const", bufs=1) as pconst, \
         tc.tile_pool(name="acts", bufs=1) as pacts, \
         tc.tile_pool(name="dram", bufs=1, space="DRAM") as pdram:

        # ---- constants / inputs resident in SBUF
        cosq = pconst.tile([128, TB], F32, name="cosq"); sync.dma_start(cosq[:], d['cosq_d'])
        sinq = pconst.tile([128, TB], F32, name="sinq"); sync.dma_start(sinq[:], d['sinq_d'])
        cosk = pconst.tile([128, T2], F32, name="cosk"); sync.dma_start(cosk[:], d['cosk_d'])
        sink = pconst.tile([128, T2], F32, name="sink"); sync.dma_start(sink[:], d['sink_d'])
        cosm = pconst.tile([128, TB], F32, name="cosm"); sync.dma_start(cosm[:], d['cosm_d'])
        sinm = pconst.tile([128, TB], F32, name="sinm"); sync.dma_start(sinm[:], d['sinm_d'])
        maskt = pconst.tile([64, 192], F32, name="maskt"); sync.dma_start(maskt[:], d['mask_d'])
        ones = pconst.tile([128, 1], F32, name="ones"); sync.dma_start(ones[:], d['ones_d'])
        onesr = pconst.tile([1, 128], F32, name="onesr"); sync.dma_start(onesr[:], d['onesr_d'])
        ident = pconst.tile([64, 64], BF16, name="ident"); sync.dma_start(ident[:], d['ident_d'])
        rv1 = pconst.tile([1, T2], F32)
        rv1b = pconst.tile([128, T2], F32)
        rv1t = pconst.tile([128, 4], F32)
        rv2b = pconst.tile([128, TB], BF16)
        eps1 = pconst.tile([1, 1], F32)
        vec.memset(eps1[:], EPS)

        # ---- persistent activations
        kTs = pacts.tile([128, MT, T2], BF16)     # roped+normed K (feature-major)
        qTs = pacts.tile([128, MT, TB], BF16)
        mkTs = pacts.tile([128, MT, TB], BF16)
        # token-major V pieces, base-0 per-batch layout [tok(64), b, feat]
        ordv2 = pacts.tile([64, B, HPC], BF16)
        beav2 = pacts.tile([64, B, HPC], BF16)
        memv2 = pacts.tile([64, B, HPC], BF16)
        oTs = pacts.tile([128, MT, TB], BF16)     # attention out (feature-major)

        # chunk-contiguous layout [k, p, t]; bf16 wire format for the AllReduce
        NCH = 1
        KCH = KT // NCH
        cc_in = pdram.tile([KT, 128, TB], BF16)
        cc_outs = [pdram.tile([KCH, 128, TB], BF16, addr_space="Shared",
                              name=f"cc_out{c}") for c in range(NCH)]

        # ================= phase 0: load x/mem, rms1 stats =================
        with tc.tile_pool(name="px", bufs=1) as px, \
             tc.tile_pool(name="pp_ss", bufs=1, space="PSUM") as pp_ss, \
             tc.tile_pool(name="psq", bufs=3) as psq:
            xt = px.tile([128, KT, T2], BF16)
            memt = px.tile([128, KT, TB], BF16)
            for c in range(4):
                kc = slice(c * 8, (c + 1) * 8)
                sync.dma_start(xt[:, kc], d['x_d'][:, kc])
                sync.dma_start(memt[:, kc], d['mem_d'][:, kc])

            ss1 = pp_ss.tile([1, T2], F32)
            for k in range(KT):
                sq = psq.tile([128, T2], F32, name="sq", tag="sq")
                vec.tensor_tensor(sq[:], xt[:, k], xt[:, k], op=ALU.mult)
                ten.matmul(ss1[:], ones[:], sq[:], start=(k == 0), stop=(k == KT - 1))
            sd1 = pconst.tile([1, T2], F32)
            sca.activation(sd1[:], ss1[:], ACTF.Sqrt, bias=eps1[0:1, 0:1], scale=1.0 / H)
            vec.reciprocal(rv1[:], sd1[:])
            bc1 = pp_ss.tile([128, T2], F32, name="bc1", tag="bc1")
            ten.matmul(bc1[:], onesr[:], rv1[:])
            vec.tensor_copy(rv1b[:], bc1[:])
            # token-major rv1 for scaling V: rv1t[p, m] = rv1[0, m*128+p]
            for m in range(4):
                sync.dma_start(rv1t[:, m:m + 1], rv1[0:1, m * 128:(m + 1) * 128])

            # ================= phase 1: V projections (token-major) ========
            with tc.tile_pool(name="pvst", bufs=1) as pvst, \
                 tc.tile_pool(name="pwv", bufs=3) as pwv, \
                 tc.tile_pool(name="pp_v", bufs=1, space="PSUM") as pp_v:
                ordv = pvst.tile([128, 2, HPC], BF16)
                beav = pvst.tile([128, 2, HPC], BF16)
                memv = pvst.tile([128, 2, HPC], BF16)
                vps = [pp_v.tile([128, HPC], F32, name=f"vps{j}", tag=f"vps{j}")
                       for j in range(4)]
                for kg in range(8):
                    wvt = pwv.tile([128, 4, HPC], BF16, name="wvt", tag="wv")
                    sync.dma_start(wvt[:], d['wv_d'][:, kg])
                    for kk in range(4):
                        k = kg * 4 + kk
                        st, sp = (k == 0), (k == KT - 1)
                        ten.matmul(vps[0][:], xt[:, k, 0:128], wvt[:, kk], start=st, stop=sp)
                        ten.matmul(vps[1][:], xt[:, k, 128:256], wvt[:, kk], start=st, stop=sp)
                        ten.matmul(vps[2][:], xt[:, k, 256:384], wvt[:, kk], start=st, stop=sp)
                        ten.matmul(vps[3][:], xt[:, k, 384:512], wvt[:, kk], start=st, stop=sp)
                for j in range(2):
                    vec.tensor_scalar_mul(ordv[:, j], vps[j][:], rv1t[:, j:j + 1])
                    vec.tensor_scalar_mul(beav[:, j], vps[2 + j][:], rv1t[:, 2 + j:3 + j])
                with tc.tile_pool(name="pwvm", bufs=3) as pwvm, \
                     tc.tile_pool(name="pp_vm", bufs=1, space="PSUM") as pp_vm:
                    mps = [pp_vm.tile([128, HPC], F32, name=f"mps{j}", tag=f"mps{j}")
                           for j in range(2)]
                    for kg in range(8):
                        wvmt = pwvm.tile([128, 4, HPC], BF16, name="wvmt", tag="wvm")
                        sync.dma_start(wvmt[:], d['wvm_d'][:, kg])
                        for kk in range(4):
                            k = kg * 4 + kk
                            st, sp = (k == 0), (k == KT - 1)
                            ten.matmul(mps[0][:], memt[:, k, 0:128], wvmt[:, kk], start=st, stop=sp)
                            ten.matmul(mps[1][:], memt[:, k, 128:256], wvmt[:, kk], start=st, stop=sp)
                    for j in range(2):
                        vec.tensor_copy(memv[:, j], mps[j][:])
                # rearrange to base-0 per-batch layout: v2[s, b, f] =
                # v[64*(b%2)+s, b//2, f]  (one SBUF->SBUF DMA each)
                for src_t, dst_t in ((ordv, ordv2), (beav, beav2), (memv, memv2)):
                    for hh in range(2):
                        sync.dma_start(
                            dst_t[:].rearrange("s (j h) f -> s j h f", h=2)[:, :, hh],
                            src_t[64 * hh:64 * (hh + 1)])

            # ========== phase 2: K/Q/MK projections + rope + normalize =====
            with tc.tile_pool(name="pwl", bufs=3) as pwl, \
                 tc.tile_pool(name="prope", bufs=2) as pr, \
                 tc.tile_pool(name="pp_k", bufs=2, space="PSUM") as pp_k:
                for m in range(MT):
                    for src, wdram, cos_t, sin_t, ncols, coloff, dst, norm in (
                            ('x', 'wk_d', cosk, sink, T2, 0, kTs, True),
                            ('x', 'wq_d', cosq, sinq, TB, TB, qTs, True),
                            ('m', 'wkm_d', cosm, sinm, TB, 0, mkTs, False)):
                        wt = pwl.tile([128, KT, 128], BF16, name="wt", tag="wl")
                        sync.dma_start(wt[:], d[wdram][:, m])
                        ps = pp_k.tile([128, ncols], F32, name="kps", tag="kps")
                        for k in range(KT):
                            rhs = xt[:, k, coloff:coloff + ncols] if src == 'x' \
                                else memt[:, k]
                            ten.matmul(ps[:], wt[:, k], rhs, start=(k == 0),
                                       stop=(k == KT - 1))
                        raw = pr.tile([128, ncols], F32, name="raw", tag="raw")
                        vec.tensor_copy(raw[:], ps[:])
                        rot = pr.tile([128, ncols], F32, name="rot", tag="rot")
                        sync.dma_start(rot[0:64, :], raw[64:128, :])
                        sync.dma_start(rot[64:128, :], raw[0:64, :])
                        t1 = pr.tile([128, ncols], F32, name="t1", tag="t1")
                        vec.tensor_tensor(t1[:], rot[:], sin_t[:], op=ALU.mult)
                        t2 = pr.tile([128, ncols], F32, name="t2", tag="t2")
                        vec.tensor_tensor(t2[:], raw[:], cos_t[:], op=ALU.mult)
                        if norm:
                            t3 = pr.tile([128, ncols], F32, name="t3", tag="t3")
                            vec.tensor_tensor(t3[:], t1[:], t2[:], op=ALU.add)
                            rvsl = rv1b[:, coloff:coloff + ncols] if ncols != T2 \
                                else rv1b[:]
                            vec.tensor_tensor(dst[:, m], t3[:], rvsl, op=ALU.mult)
                        else:
                            vec.tensor_tensor(dst[:, m], t1[:], t2[:], op=ALU.add)

        # ====== phases 3+4: attention (head-outer), per-head Wo accumulate,
        # ====== chunked AllReduce overlapping the attention tail
        with tc.tile_pool(name="acts2", bufs=1) as pacts2:
            xacc = pacts2.tile([128, KT, TB], F32)    # Wo partial + beacon/8
            ccst = pacts2.tile([128, KT, TB], BF16)   # bf16 staging for AR input
            xbs = pacts2.tile([128, KT, TB], BF16)    # post-AR residual
            h2s = pacts2.tile([128, KT, TB], BF16)
            h3s = pacts2.tile([128, GT, TB], BF16)
            sync.dma_start(xacc[:], d['b8_d'])        # init with beacon/8
            att_block = [
                tc.tile_pool(name="patt", bufs=3),
                tc.tile_pool(name="pwo", bufs=1),
                tc.tile_pool(name="pp_sc", bufs=2, space="PSUM"),
                tc.tile_pool(name="pp_tr", bufs=1, space="PSUM"),
                tc.tile_pool(name="pp_ot", bufs=1, space="PSUM"),
                tc.tile_pool(name="pp_wo", bufs=2, space="PSUM"),
            ]
            pa, pwo, pp_sc, pp_tr, pp_ot, pp_wo = [p.__enter__() for p in att_block]
            wo_all = pwo.tile([128, KT, MT, 128], BF16)
            sync.dma_start(wo_all[:], d['wo_d'])
            for h in range(MT):
                hc = 128 * h
                for b in range(B):
                    bo = 64 * b
                    sc_ps = pp_sc.tile([64, 192], F32, name="sc_ps", tag="sc")
                    q_ap = qTs[:, h, bo:bo + 64]
                    ten.matmul(sc_ps[:, 0:64], q_ap, mkTs[:, h, bo:bo + 64])
                    ten.matmul(sc_ps[:, 64:128], q_ap, kTs[:, h, bo:bo + 64])
                    ten.matmul(sc_ps[:, 128:192], q_ap, kTs[:, h, TB + bo:TB + bo + 64])
                    sc = pa.tile([64, 192], F32, name="sc", tag="scs")
                    vec.scalar_tensor_tensor(sc[:], sc_ps[:], SCALE, maskt[:],
                                             op0=ALU.mult, op1=ALU.add)
                    mx = pa.tile([64, 1], F32, name="mx", tag="mx")
                    vec.reduce_max(mx[:], sc[:], axis=AX.X)
                    mxn = pa.tile([64, 1], F32, name="mxn", tag="mxn")
                    vec.tensor_scalar_mul(mxn[:], mx[:], -1.0)
                    pex = pa.tile([64, 192], F32, name="pex", tag="pex")
                    rs = pa.tile([64, 1], F32, name="rs", tag="rs")
                    sca.activation(pex[:], sc[:], ACTF.Exp, bias=mxn[:, 0:1],
                                   accum_out=rs[:, 0:1])
                    rr = pa.tile([64, 1], F32, name="rr", tag="rr")
                    vec.reciprocal(rr[:], rs[:])
                    pbf = pa.tile([64, 192], BF16, name="pbf", tag="pbf")
                    vec.tensor_scalar_mul(pbf[:], pex[:], rr[:, 0:1])
                    trm = pp_tr.tile([64, 64], BF16, name="trm", tag="trm")
                    ten.transpose(trm[:], pbf[:, 0:64], ident[:])
                    tro = pp_tr.tile([64, 64], BF16, name="tro", tag="tro")
                    ten.transpose(tro[:], pbf[:, 64:128], ident[:])
                    trb = pp_tr.tile([64, 64], BF16, name="trb", tag="trb")
                    ten.transpose(trb[:], pbf[:, 128:192], ident[:])
                    ptm = pa.tile([64, 64], BF16, name="ptm", tag="ptm")
                    vec.tensor_copy(ptm[:], trm[:])
                    pto = pa.tile([64, 64], BF16, name="pto", tag="pto")
                    vec.tensor_copy(pto[:], tro[:])
                    ptb = pa.tile([64, 64], BF16, name="ptb", tag="ptb")
                    vec.tensor_copy(ptb[:], trb[:])
                    o_ps = pp_ot.tile([128, 64], F32, name="o_ps", tag="ops")
                    ten.matmul(o_ps[:], memv2[:, b, hc:hc + 128],
                               ptm[:], start=True, stop=False)
                    ten.matmul(o_ps[:], ordv2[:, b, hc:hc + 128],
                               pto[:], start=False, stop=False)
                    ten.matmul(o_ps[:], beav2[:, b, hc:hc + 128],
                               ptb[:], start=False, stop=True)
                    vec.tensor_copy(oTs[:, h, bo:bo + 64], o_ps[:])
                # this head's Wo contribution into xacc (all 32 m-tiles)
                for m in range(KT):
                    wps = pp_wo.tile([128, TB], F32, name="wps", tag="wps")
                    ten.matmul(wps[:], wo_all[:, m, h, :], oTs[:, h, :])
                    if h < MT - 1:
                        vec.tensor_tensor(xacc[:, m], wps[:], xacc[:, m], op=ALU.add)
                    else:
                        vec.tensor_tensor(ccst[:, m], wps[:], xacc[:, m], op=ALU.add)
                        sync.dma_start(cc_in[m], ccst[:, m])
                        if (m + 1) % KCH == 0:
                            c = m // KCH
                            cs = slice(c * KCH, (c + 1) * KCH)
                            gps.collective_compute(
                                "AllReduce", ALU.add,
                                replica_groups=[list(range(NCORES))],
                                ins=[cc_in[cs].opt()], outs=[cc_outs[c].opt()])
                            sync.dma_start(
                                xbs[:, cs],
                                cc_outs[c][:].rearrange("k p t -> p k t"))
                            sync.dma_start(d['oxb_d'][:, cs], xbs[:, cs])
            for p in reversed(att_block):
                p.__exit__(None, None, None)

            # =============== phase 5: rms2 + h2 (chunk-pipelined) ==========
            with tc.tile_pool(name="psq2", bufs=3) as psq2, \
                 tc.tile_pool(name="pp_s2", bufs=1, space="PSUM") as pp_s2:
                ss2 = pp_s2.tile([1, TB], F32)
                for k in range(KT):
                    sq2 = psq2.tile([128, TB], F32, name="sq2", tag="sq2")
                    vec.tensor_tensor(sq2[:], xbs[:, k], xbs[:, k], op=ALU.mult)
                    ten.matmul(ss2[:], ones[:], sq2[:], start=(k == 0),
                               stop=(k == KT - 1))
                sd2 = pconst.tile([1, TB], F32)
                sca.activation(sd2[:], ss2[:], ACTF.Sqrt, bias=eps1[0:1, 0:1],
                               scale=1.0 / H)
                rv2 = pconst.tile([1, TB], F32)
                vec.reciprocal(rv2[:], sd2[:])
                bc2 = pp_s2.tile([128, TB], F32, name="bc2", tag="bc2")
                ten.matmul(bc2[:], onesr[:], rv2[:])
                vec.tensor_copy(rv2b[:], bc2[:])
                for k in range(KT):
                    vec.tensor_tensor(h2s[:, k], xbs[:, k], rv2b[:], op=ALU.mult)

            # =============== phase 6: MLP gate/up ============================
            with tc.tile_pool(name="pwg", bufs=6) as pwg, \
                 tc.tile_pool(name="pgu", bufs=2) as pgu, \
                 tc.tile_pool(name="pp_gu", bufs=2, space="PSUM") as pp_gu:
                for m in range(GT):
                    wgt = pwg.tile([128, KT, 128], BF16, name="wgt", tag="wgu")
                    sync.dma_start(wgt[:], d['wg_d'][:, m])
                    wut = pwg.tile([128, KT, 128], BF16, name="wut", tag="wgu")
                    sync.dma_start(wut[:], d['wu_d'][:, m])
                    gps_t = pp_gu.tile([128, TB], F32, name="gps_t", tag="gps")
                    ups_t = pp_gu.tile([128, TB], F32, name="ups_t", tag="ups")
                    for k in range(KT):
                        st, sp = (k == 0), (k == KT - 1)
                        ten.matmul(gps_t[:], wgt[:, k], h2s[:, k], start=st, stop=sp)
                        ten.matmul(ups_t[:], wut[:, k], h2s[:, k], start=st, stop=sp)
                    # silu(g) * u = g * sigmoid(g) * u  (sim lacks Silu)
                    gsig = pgu.tile([128, TB], F32, name="gsig", tag="gsig")
                    sca.activation(gsig[:], gps_t[:], ACTF.Sigmoid)
                    gact = pgu.tile([128, TB], BF16, name="gact", tag="gact")
                    vec.tensor_tensor(gact[:], gps_t[:], gsig[:], op=ALU.mult)
                    ubf = pgu.tile([128, TB], BF16, name="ubf", tag="ubf")
                    vec.tensor_copy(ubf[:], ups_t[:])
                    vec.tensor_tensor(h3s[:, m], gact[:], ubf[:], op=ALU.mult)

            # =============== phase 7: MLP down (partial) =====================
            with tc.tile_pool(name="pwd", bufs=3) as pwd, \
                 tc.tile_pool(name="pds", bufs=3) as pds, \
                 tc.tile_pool(name="pp_d", bufs=2, space="PSUM") as pp_d:
                for m in range(KT):
                    wdt = pwd.tile([128, GT, 128], BF16, name="wdt", tag="wd")
                    sync.dma_start(wdt[:], d['wd_d'][:, m])
                    dps = pp_d.tile([128, TB], F32, name="dps", tag="dps")
                    for k in range(GT):
                        ten.matmul(dps[:], wdt[:, k], h3s[:, k], start=(k == 0),
                                   stop=(k == GT - 1))
                    dst = pds.tile([128, TB], F32, name="dst", tag="dst")
                    vec.tensor_copy(dst[:], dps[:])
                    sync.dma_start(d['od_d'][:, m], dst[:])


# ======================= host-side prep / unshard ==========================

def _bf16(x):
    return np.ascontiguousarray(np.asarray(x, np.float32).astype(ml_dtypes.bfloat16))


def _fm_tiles(fm):
    """[H, T] feature-major -> [128, KT, T] partition-major tiles."""
    T = fm.shape[1]
    return np.ascontiguousarray(fm.reshape(KT, 128, T).transpose(1, 0, 2))


def _lhsT_tiles(w, mt):
    """[H_in, M] weight -> [128, mt, H_in//128, 128] lhsT tiles."""
    hi, mo = w.shape
    kt = hi // 128
    return np.ascontiguousarray(
        w.reshape(kt, 128, mt, 128).transpose(1, 2, 0, 3))


def _host_prep(inputs):
    ids = np.asarray(inputs['input_ids'])
    embed = np.asarray(inputs['embed'], np.float32)
    beacon = np.asarray(inputs['beacon'], np.float32)
    memory = np.asarray(inputs['memory'], np.float32)
    hs = embed[ids]                                     # [B,S,H]
    x_fm = np.concatenate([hs.reshape(TB, H).T, beacon.reshape(TB, H).T], axis=1)
    mem_fm = memory[0:B].reshape(TB, H).T
    b8_fm = (beacon.reshape(TB, H).T / 8.0).astype(np.float32)

    inv = 1.0 / (10000.0 ** (np.arange(0, HD, 2, dtype=np.float32) / HD))
    f = np.outer(np.arange(3 * S, dtype=np.float32), inv)
    emb = np.concatenate([f, f], axis=-1)               # [3S, HD]
    cos = np.cos(emb).astype(np.float32)
    sin = np.sin(emb).astype(np.float32)

    def fm_tab(p0):
        c = cos[p0:p0 + S].T                            # [HD, S]
        s = sin[p0:p0 + S].T
        s = np.concatenate([-s[:64], s[64:]], axis=0)   # sign-baked for rotate64
        return np.tile(c, (1, B)), np.tile(s, (1, B))
    cosm_t, sinm_t = fm_tab(0)
    coso_t, sino_t = fm_tab(S)
    cosb_t, sinb_t = fm_tab(2 * S)
    cosk_t = np.concatenate([coso_t, cosb_t], axis=1)
    sink_t = np.concatenate([sino_t, sinb_t], axis=1)

    # mask tile columns are [mem(64) | ord(64) | bea(64)] = 192, additive
    maskadd = np.zeros((S, 3 * S), np.float32)
    ii = np.arange(S)[:, None]
    jj = np.arange(S)[None, :]
    bea_block = np.where(jj > ii, np.float32(-1e30), np.float32(0.0))
    maskadd[:, 2 * S:] = bea_block

    ln1 = np.asarray(inputs['ln1'], np.float32)[0]
    ln2 = np.asarray(inputs['ln2'], np.float32)[0]
    Wq = np.asarray(inputs['Wq'], np.float32)[0] * ln1[:, None]
    Wk = np.asarray(inputs['Wk'], np.float32)[0] * ln1[:, None]
    Wv = np.asarray(inputs['Wv'], np.float32)[0] * ln1[:, None]
    Wkm = np.asarray(inputs['Wkm'], np.float32)[0]
    Wvm = np.asarray(inputs['Wvm'], np.float32)[0]
    Wo = np.asarray(inputs['Wo'], np.float32)[0]
    Wg = np.asarray(inputs['Wg'], np.float32)[0] * ln2[:, None]
    Wu = np.asarray(inputs['Wu'], np.float32)[0] * ln2[:, None]
    Wd = np.asarray(inputs['Wd'], np.float32)[0]

    shared = {
        'x_d': _fm_tiles(_bf16(x_fm)),
        'mem_d': _fm_tiles(_bf16(mem_fm)),
        'b8_d': np.ascontiguousarray(_fm_tiles(b8_fm)),
        'cosq_d': np.ascontiguousarray(cosb_t), 'sinq_d': np.ascontiguousarray(sinb_t),
        'cosk_d': np.ascontiguousarray(cosk_t), 'sink_d': np.ascontiguousarray(sink_t),
        'cosm_d': np.ascontiguousarray(cosm_t), 'sinm_d': np.ascontiguousarray(sinm_t),
        'mask_d': maskadd,
        'ones_d': np.ones((128, 1), np.float32),
        'onesr_d': np.ones((1, 128), np.float32),
        'ident_d': _bf16(np.eye(64, dtype=np.float32)),
    }

    in_maps = []
    for c in range(NCORES):
        sl = slice(c * HPC, (c + 1) * HPC)
        isl = slice(c * IPC, (c + 1) * IPC)
        wg = np.zeros((H, IPC_PAD), np.float32); wg[:, :IPC] = Wg[:, isl]
        wu = np.zeros((H, IPC_PAD), np.float32); wu[:, :IPC] = Wu[:, isl]
        wd = np.zeros((IPC_PAD, H), np.float32); wd[:IPC] = Wd[isl]
        wv_c = _bf16(Wv[:, sl])
        wvm_c = _bf16(Wvm[:, sl])
        m = dict(shared)
        m['wq_d'] = _lhsT_tiles(_bf16(Wq[:, sl]), MT)
        m['wk_d'] = _lhsT_tiles(_bf16(Wk[:, sl]), MT)
        m['wkm_d'] = _lhsT_tiles(_bf16(Wkm[:, sl]), MT)
        m['wv_d'] = np.ascontiguousarray(
            wv_c.reshape(8, 4, 128, HPC).transpose(2, 0, 1, 3))
        m['wvm_d'] = np.ascontiguousarray(
            wvm_c.reshape(8, 4, 128, HPC).transpose(2, 0, 1, 3))
        m['wo_d'] = np.ascontiguousarray(
            _bf16(Wo[sl, :]).reshape(MT, 128, KT, 128).transpose(1, 2, 0, 3))
        m['wg_d'] = _lhsT_tiles(_bf16(wg), GT)
        m['wu_d'] = _lhsT_tiles(_bf16(wu), GT)
        m['wd_d'] = np.ascontiguousarray(
            _bf16(wd).reshape(GT, 128, KT, 128).transpose(1, 2, 0, 3))
        in_maps.append(m)
    return in_maps, beacon


def kernel(**inputs):
    in_maps, beacon = _host_prep(inputs)
    if 'nc' not in _CACHE:
        _CACHE['nc'] = _build_nc()
    nc = _CACHE['nc']
    res = run_bass_kernel_spmd(nc, in_maps, core_ids=list(range(NCORES)))
    _CACHE['last_res'] = res
    results = res.results
    xb = results[0]['oxb_d'].astype(np.float32)          # [128, KT, TB]
    dsum = np.sum([r['od_d'] for r in results], axis=0, dtype=np.float32)
    bs_fm = (xb + dsum).transpose(1, 0, 2).reshape(H, TB)  # [H, 256]
    out = np.empty((2 * B, S, H), np.float32)
    out[:B] = np.asarray(inputs['beacon'], np.float32)
    out[B:] = bs_fm.T.reshape(B, S, H)
    return out
